# revision 13
# baseline (speedup 1.0000x reference)
"""Trainium2 Bass kernel for nn_AttentionBlock: LN -> QKV -> softmax(K@Q^T) attn
-> residual -> LN -> 3-layer MLP -> residual.

Data-parallel over batch: B=8 batch elements, one per NeuronCore. No collectives.

Per-core layout strategy:
  - fp32 token-major backbone (x, x2, LN stats), f16 matmul operands.
  - Feature-major ("transposed") f16 activations feed the TensorEngine; the
    transposes ride on the DMA xbar (dma_start_transpose), keeping PE free.
  - Q/K projections emit feature-major (q^T, k^T); V emits token-major into a
    65-stride layout whose 65th column is ones, so each AV matmul also
    produces the softmax denominator (row 64 of PSUM).
  - Scores S^T = K'^T-chunk.T @ Q'^T are K=64 contractions, row-packed two
    heads per pass via tile_position (0,0)/(64,0).
  - exp on ScalarE with scale=1/8 (the 1/sqrt(dh)) and bias=-log(32) to keep
    f16-staged exp values comfortably in range (softmax is shift-invariant).
"""
import numpy as np
import concourse.bass as bass
import concourse.tile as tile
from concourse import bacc, mybir
from concourse.bass_utils import run_bass_kernel_spmd

F16 = mybir.dt.float16
F32 = mybir.dt.float32
AX = mybir.AxisListType
OP = mybir.AluOpType
AF = mybir.ActivationFunctionType

B = 8
EPS = 1e-5
LOG_SHIFT = float(np.log(32.0))


def build_attention_block(T=1024, D=1024, n_devices=8, debug_taps=False):
    DH = 64
    H = D // DH          # heads
    NF = D // 128        # feature chunks (also head-pairs)
    NT = T // 128        # token chunks
    TW = T // 2          # half-of-tokens tile width (<= 512 for one PSUM bank)
    assert TW <= 512

    nc = bacc.Bacc("TRN2", target_bir_lowering=False, debug=False,
                   num_devices=n_devices)

    x_d = nc.declare_dram_parameter("x", [T, D], F32, isOutput=False)
    w_d = {}
    b_d = {}
    for nm in ("Wq", "Wk", "Wv", "W0", "W1", "W2"):
        w_d[nm] = nc.declare_dram_parameter(nm, [D, D], F32, isOutput=False)
    for nm in ("bq", "bk", "bv", "b0", "b1", "b2", "g1", "be1", "g2", "be2"):
        b_d[nm] = nc.declare_dram_parameter(nm, [D], F32, isOutput=False)
    out_d = nc.declare_dram_parameter("out", [T, D], F32, isOutput=True)
    dbg = {}
    if debug_taps:
        for nm, shape in [("xnT", [128, D // 128, T]), ("qT", [128, D // 128, T]),
                          ("kT", [128, D // 128, T]), ("vpad", [128, T // 128, (D // 64) * 65]),
                          ("y_tm", [128, T // 128, D // 128, 128]), ("rs", [16, T]),
                          ("x2", [T, D]), ("xn2T", [128, D // 128, T]),
                          ("h1T", [128, D // 128, T]), ("h3T", [128, D // 128, T])]:
            dbg[nm] = nc.declare_dram_parameter("dbg_" + nm, shape, F32, isOutput=True)

    with tile.TileContext(nc) as tc:
        with (
            tc.tile_pool(name="wp", bufs=2) as wp,            # (128, NF, D) f16 weights
            tc.tile_pool(name="bigp", bufs=4) as bigp,        # (128, NF, T)-ish f16
            tc.tile_pool(name="vp", bufs=1) as vp,            # vpad
            tc.tile_pool(name="ep", bufs=3) as ep,            # e^T chunks
            tc.tile_pool(name="up", bufs=3) as up,            # token-major u tiles
            tc.tile_pool(name="yp", bufs=3) as yp,            # yT pair tiles
            tc.tile_pool(name="xp", bufs=3) as xp,            # x fp32 tiles
            tc.tile_pool(name="x2p", bufs=NT) as x2p,         # x2 f16 tiles (all live)
            tc.tile_pool(name="scrp", bufs=2) as scrp,        # ACT accum scratch
            tc.tile_pool(name="outp", bufs=3) as outp,        # out fp32 staging
            tc.tile_pool(name="smallp", bufs=24) as smallp,   # stats & misc small
            tc.tile_pool(name="constp", bufs=1) as constp,    # per-partition consts
            tc.tile_pool(name="psp", bufs=4, space="PSUM") as psp,
        ):
            def dump(nm, src_ap):
                if not debug_taps:
                    return
                flat_n = 1
                for d_ in src_ap.shape[1:]:
                    flat_n *= d_
                stg = smallp.tile([src_ap.shape[0], flat_n], F32, tag="dump_" + nm,
                                  name="dump_" + nm, bufs=1)
                flat_src = src_ap
                if len(src_ap.shape) == 3:
                    flat_src = src_ap.rearrange("p a b -> p (a b)")
                elif len(src_ap.shape) == 4:
                    flat_src = src_ap.rearrange("p a b c -> p (a b c)")
                nc.vector.tensor_copy(stg[:], flat_src)
                dst = dbg[nm].ap()
                if len(dst.shape) == 3:
                    dst = dst.rearrange("p a b -> p (a b)")
                elif len(dst.shape) == 4:
                    dst = dst.rearrange("p a b c -> p (a b c)")
                nc.sync.dma_start(dst, stg[:])

            # ---------------- constant / vector loads ----------------
            bsb = {}
            for nm in ("bq", "bk", "b0", "b1", "b2", "g1", "be1", "g2", "be2"):
                t = constp.tile([128, NF], F32, tag=nm)
                nc.sync.dma_start(t[:], b_d[nm].ap().rearrange("(c p) -> p c", p=128))
                bsb[nm] = t
            # bv broadcast across partitions (token-major use)
            bv_row = constp.tile([1, D], F32, tag="bv_row")
            nc.sync.dma_start(bv_row[:], b_d["bv"].ap().rearrange("(o d) -> o d", o=1))
            bvb = constp.tile([128, D], F32, tag="bvb")
            nc.gpsimd.partition_broadcast(bvb[:], bv_row[:])
            # scalar bias constants for activations
            eps_t = constp.tile([128, 1], F32, tag="eps")
            nc.gpsimd.memset(eps_t[:], EPS)
            nls_t = constp.tile([128, 1], F32, tag="nls")
            nc.gpsimd.memset(nls_t[:], -LOG_SHIFT)

            # ---------------- weight loads (f32 -> f16 SWDGE cast) ----------------
            def load_weight(nm):
                t = wp.tile([128, NF, D], F16, tag="w")
                nc.gpsimd.dma_start(t[:], w_d[nm].ap().rearrange("(c p) n -> p c n", p=128))
                return t

            wq_sb = load_weight("Wq")
            wk_sb = load_weight("Wk")

            # ---------------- LayerNorm helper ----------------
            def layer_norm_to_uT(x_tiles, uT_big, g_t, be_t, xnT_big):
                """x_tiles: NT token-major tiles -> xnT_big (128, NF, T) f16."""
                for i in range(NT):
                    xi = x_tiles[i]
                    scr = scrp.tile([128, D], F16, tag="scr")
                    ssum = smallp.tile([128, 1], F32, tag="st")
                    nc.scalar.activation(scr[:], xi[:], AF.Identity, accum_out=ssum[:])
                    scr2 = scrp.tile([128, D], F16, tag="scr")
                    ssq = smallp.tile([128, 1], F32, tag="st")
                    nc.scalar.activation(scr2[:], xi[:], AF.Square, accum_out=ssq[:])
                    mean = smallp.tile([128, 1], F32, tag="st")
                    nc.vector.tensor_scalar_mul(mean[:], ssum[:], 1.0 / D)
                    m2 = smallp.tile([128, 1], F32, tag="st")
                    nc.vector.tensor_tensor(out=m2[:], in0=mean[:], in1=mean[:], op=OP.mult)
                    v0 = smallp.tile([128, 1], F32, tag="st")
                    nc.vector.tensor_scalar_mul(v0[:], ssq[:], 1.0 / D)
                    var = smallp.tile([128, 1], F32, tag="st")
                    nc.vector.tensor_tensor(out=var[:], in0=v0[:], in1=m2[:], op=OP.subtract)
                    std = smallp.tile([128, 1], F32, tag="st")
                    nc.scalar.activation(std[:], var[:], AF.Sqrt, bias=eps_t[:])
                    rstd = smallp.tile([128, 1], F32, tag="st")
                    nc.vector.reciprocal(rstd[:], std[:])
                    u = up.tile([128, D], F16, tag="u")
                    nc.vector.tensor_scalar(out=u[:], in0=xi[:], scalar1=mean[:],
                                            scalar2=rstd[:], op0=OP.subtract, op1=OP.mult)
                    nc.sync.dma_start_transpose(uT_big[:, :, i * 128:(i + 1) * 128], u[:])
                # apply gain/bias per feature chunk (per-partition scalars, 4x DVE)
                for c in range(NF):
                    nc.vector.tensor_scalar(out=xnT_big[:, c, :], in0=uT_big[:, c, :],
                                            scalar1=g_t[:, c:c + 1], scalar2=be_t[:, c:c + 1],
                                            op0=OP.mult, op1=OP.add)

            # ---------------- LN1 ----------------
            x_tiles = []
            for i in range(NT):
                xi = xp.tile([128, D], F32, tag="x")
                nc.sync.dma_start(xi[:], x_d[i * 128:(i + 1) * 128, :])
                x_tiles.append(xi)
            uT_big = bigp.tile([128, NF, T], F16, tag="big")
            xnT_big = bigp.tile([128, NF, T], F16, tag="big")
            layer_norm_to_uT(x_tiles, uT_big, bsb["g1"], bsb["be1"], xnT_big)

            dump("xnT", xnT_big[:])

            # ---------------- QKV projections ----------------
            qT_big = bigp.tile([128, NF, T], F16, tag="big")
            kT_big = bigp.tile([128, NF, T], F16, tag="big")

            def proj_feature_major(w_sb, bias_t, dst_big, m, th):
                ps = psp.tile([128, TW], F32, tag="mm")
                for kc in range(NF):
                    nc.tensor.matmul(ps[:], w_sb[:, kc, m * 128:(m + 1) * 128],
                                     xnT_big[:, kc, th * TW:(th + 1) * TW],
                                     start=(kc == 0), stop=(kc == NF - 1))
                nc.vector.tensor_scalar(out=dst_big[:, m, th * TW:(th + 1) * TW],
                                        in0=ps[:], scalar1=bias_t[:, m:m + 1],
                                        scalar2=None, op0=OP.add)

            # interleave K/Q so attention can start early
            for m in range(NF):
                for th in range(2):
                    proj_feature_major(wk_sb, bsb["bk"], kT_big, m, th)
                    proj_feature_major(wq_sb, bsb["bq"], qT_big, m, th)

            wv_sb = load_weight("Wv")
            vpad = vp.tile([128, NT, H * 65], F16, tag="vpad")
            nc.vector.memset(vpad[:, :, :].rearrange("p c (h o) -> p c h o", o=65)[:, :, :, 64], 1.0)
            for i in range(NT):
                for dh in range(2):
                    hph = (D // 2) // 64  # heads per half
                    ps = psp.tile([128, D // 2], F32, tag="mm")
                    for kc in range(NF):
                        nc.tensor.matmul(ps[:], xnT_big[:, kc, i * 128:(i + 1) * 128],
                                         wv_sb[:, kc, dh * (D // 2):(dh + 1) * (D // 2)],
                                         start=(kc == 0), stop=(kc == NF - 1))
                    dst = vpad[:, i, :].rearrange("p (h o) -> p h o", o=65)[:, dh * hph:(dh + 1) * hph, 0:64]
                    nc.vector.tensor_tensor(
                        out=dst,
                        in0=ps[:].rearrange("p (h q) -> p h q", q=64),
                        in1=bvb[:, dh * (D // 2):(dh + 1) * (D // 2)].rearrange("p (h q) -> p h q", q=64),
                        op=OP.add)

            dump("qT", qT_big[:])
            dump("kT", kT_big[:])
            dump("vpad", vpad[:])

            # prefetch MLP weights during attention
            w0_sb = load_weight("W0")

            # ---------------- attention ----------------
            rs_sb = smallp.tile([16, T], F16, tag="rs", bufs=1)
            if H < 16:
                nc.gpsimd.memset(rs_sb[:], 1.0)
            y_tm_big = bigp.tile([128, NT, NF, 128], F16, tag="big")
            for d in range(NF):
                y_ps = [[psp.tile([65, TW], F32, tag="y", name=f"y_ps_{d}_{hh}_{th}")
                         for th in range(2)] for hh in range(2)]
                for s in range(NT):
                    e_t = [ep.tile([128, T], F16, tag="e", name=f"e_{d}_{s}_{hh}") for hh in range(2)]
                    for th in range(2):
                        for hh, base in ((0, 0), (1, 64)):
                            sc = psp.tile([128, TW], F32, tag="mm")
                            nc.tensor.matmul(sc[:],
                                             qT_big[base:base + 64, d, s * 128:(s + 1) * 128],
                                             kT_big[base:base + 64, d, th * TW:(th + 1) * TW],
                                             start=True, stop=True,
                                             tile_position=(base, 0))
                            nc.scalar.activation(e_t[hh][:, th * TW:(th + 1) * TW], sc[:],
                                                 AF.Exp, bias=nls_t[:], scale=0.125)
                    for hh in range(2):
                        h = 2 * d + hh
                        for th in range(2):
                            nc.tensor.matmul(y_ps[hh][th][:],
                                             vpad[:, s, h * 65:h * 65 + 65],
                                             e_t[hh][:, th * TW:(th + 1) * TW],
                                             start=(s == 0), stop=(s == NT - 1))
                # extract rowsums + y^T
                yT_d = yp.tile([128, T], F16, tag="yT")
                for hh in range(2):
                    h = 2 * d + hh
                    for th in range(2):
                        stg = smallp.tile([1, TW], F16, tag="stg", bufs=4)
                        nc.vector.tensor_copy(stg[:], y_ps[hh][th][64:65, :])
                        nc.sync.dma_start(rs_sb[h:h + 1, th * TW:(th + 1) * TW], stg[:])
                        nc.vector.tensor_copy(yT_d[hh * 64:(hh + 1) * 64, th * TW:(th + 1) * TW],
                                              y_ps[hh][th][0:64, :])
                nc.sync.dma_start_transpose(y_tm_big[:, :, d, :], yT_d[:])

            dump("y_tm", y_tm_big[:])
            dump("rs", rs_sb[:])
            rsT = smallp.tile([128, NT, 16], F16, tag="rsT", bufs=1)
            nc.sync.dma_start_transpose(rsT[:], rs_sb[:])

            # ---------------- residual + LN2 ----------------
            x2_tiles = []
            uT2_big = bigp.tile([128, NF, T], F16, tag="big")
            xn2T_big = bigp.tile([128, NF, T], F16, tag="big")
            for i in range(NT):
                xi = xp.tile([128, D], F32, tag="x")
                nc.sync.dma_start(xi[:], x_d[i * 128:(i + 1) * 128, :])
                recip = smallp.tile([128, 16], F32, tag="recip", bufs=2)
                nc.vector.reciprocal(recip[:, 0:H], rsT[:, i, 0:H])
                rfull = smallp.tile([128, H, 64], F32, tag="rfull", bufs=2)
                rsrc = recip[:, 0:H].rearrange("p (h o) -> p h o", o=1)
                rsrc = bass.AP(rsrc.tensor, rsrc.offset, [rsrc.ap[0], rsrc.ap[1], [0, 64]])
                nc.vector.tensor_copy(rfull[:], rsrc)
                ysc = outp.tile([128, D], F32, tag="ysc", bufs=2)
                nc.vector.tensor_tensor(out=ysc[:],
                                        in0=y_tm_big[:, i, :, :].rearrange("p c q -> p (c q)"),
                                        in1=rfull[:].rearrange("p h o -> p (h o)"),
                                        op=OP.mult)
                x2i = x2p.tile([128, D], F16, tag="x2")
                nc.vector.tensor_tensor(out=x2i[:], in0=xi[:], in1=ysc[:], op=OP.add)
                x2_tiles.append(x2i)
            if debug_taps:
                for i in range(NT):
                    stgx = smallp.tile([128, D], F32, tag="dump_x2", name=f"dump_x2_{i}", bufs=2)
                    nc.vector.tensor_copy(stgx[:], x2_tiles[i][:])
                    nc.sync.dma_start(dbg["x2"][i * 128:(i + 1) * 128, :], stgx[:])
            layer_norm_to_uT(x2_tiles, uT2_big, bsb["g2"], bsb["be2"], xn2T_big)

            # ---------------- MLP ----------------
            h1T_big = bigp.tile([128, NF, T], F16, tag="big")
            h2T_big = bigp.tile([128, NF, T], F16, tag="big")
            h3T_big = bigp.tile([128, NF, T], F16, tag="big")

            def mlp_layer(w_sb, bias_t, src_big, dst_big, relu):
                for m in range(NF):
                    for th in range(2):
                        ps = psp.tile([128, TW], F32, tag="mm")
                        for kc in range(NF):
                            nc.tensor.matmul(ps[:], w_sb[:, kc, m * 128:(m + 1) * 128],
                                             src_big[:, kc, th * TW:(th + 1) * TW],
                                             start=(kc == 0), stop=(kc == NF - 1))
                        if relu:
                            nc.vector.tensor_scalar(out=dst_big[:, m, th * TW:(th + 1) * TW],
                                                    in0=ps[:], scalar1=bias_t[:, m:m + 1],
                                                    scalar2=0.0, op0=OP.add, op1=OP.max)
                        else:
                            nc.vector.tensor_scalar(out=dst_big[:, m, th * TW:(th + 1) * TW],
                                                    in0=ps[:], scalar1=bias_t[:, m:m + 1],
                                                    scalar2=None, op0=OP.add)

            w1_sb = load_weight("W1")
            mlp_layer(w0_sb, bsb["b0"], xn2T_big, h1T_big, relu=True)
            w2_sb = load_weight("W2")
            mlp_layer(w1_sb, bsb["b1"], h1T_big, h2T_big, relu=True)
            mlp_layer(w2_sb, bsb["b2"], h2T_big, h3T_big, relu=False)

            dump("xn2T", xn2T_big[:])
            dump("h1T", h1T_big[:])
            dump("h3T", h3T_big[:])
            h3_tm_big = bigp.tile([128, NT, NF, 128], F16, tag="big")
            for m in range(NF):
                nc.sync.dma_start_transpose(h3_tm_big[:, :, m, :], h3T_big[:, m, :])

            # ---------------- final residual + store ----------------
            for i in range(NT):
                osb = outp.tile([128, D], F32, tag="osb")
                nc.vector.tensor_tensor(out=osb[:], in0=x2_tiles[i][:],
                                        in1=h3_tm_big[:, i, :, :].rearrange("p c q -> p (c q)"),
                                        op=OP.add)
                nc.sync.dma_start(out_d[i * 128:(i + 1) * 128, :], osb[:])

    nc.compile()
    return nc


_NC_CACHE = {}


def _get_nc():
    if "nc" not in _NC_CACHE:
        _NC_CACHE["nc"] = build_attention_block(1024, 1024, 8)
    return _NC_CACHE["nc"]


def kernel(**inputs):
    nc = _get_nc()
    names = ["Wq", "bq", "Wk", "bk", "Wv", "bv", "g1", "be1", "g2", "be2",
             "W0", "b0", "W1", "b1", "W2", "b2"]
    shared = {nm: np.ascontiguousarray(np.asarray(inputs[nm], dtype=np.float32))
              for nm in names}
    x = np.asarray(inputs["x"], dtype=np.float32)
    in_maps = [dict(shared, x=np.ascontiguousarray(x[b])) for b in range(B)]
    res = run_bass_kernel_spmd(nc, in_maps, core_ids=list(range(B)))
    return np.stack([res.results[b]["out"] for b in range(B)], axis=0)


# revision 16
# speedup vs baseline: 1.2584x; 1.2584x over previous
"""Trainium2 Bass kernel for nn_AttentionBlock: LN -> QKV -> softmax(K@Q^T) attn
-> residual -> LN -> 3-layer MLP -> residual.

Data-parallel over batch: B=8 batch elements, one per NeuronCore. No collectives.

Per-core layout strategy:
  - fp32 token-major backbone (x, x2, LN stats), f16 matmul operands.
  - Feature-major ("transposed") f16 activations feed the TensorEngine; the
    transposes ride on the DMA xbar (dma_start_transpose), keeping PE free.
  - Q/K projections emit feature-major (q^T, k^T); V emits token-major into a
    65-stride layout whose 65th column is ones, so each AV matmul also
    produces the softmax denominator (row 64 of PSUM).
  - Scores S^T = K'^T-chunk.T @ Q'^T are K=64 contractions, row-packed two
    heads per pass via tile_position (0,0)/(64,0).
  - exp on ScalarE with scale=1/8 (the 1/sqrt(dh)) and bias=-log(32) to keep
    f16-staged exp values comfortably in range (softmax is shift-invariant).
"""
import numpy as np
import concourse.bass as bass
import concourse.tile as tile
from concourse import bacc, mybir
from concourse.bass_utils import run_bass_kernel_spmd

F16 = mybir.dt.float16
F32 = mybir.dt.float32
AX = mybir.AxisListType
OP = mybir.AluOpType
AF = mybir.ActivationFunctionType

B = 8
EPS = 1e-5
LOG_SHIFT = float(np.log(32.0))


def build_attention_block(T=1024, D=1024, n_devices=8, debug_taps=False):
    DH = 64
    H = D // DH          # heads
    NF = D // 128        # feature chunks (also head-pairs)
    NT = T // 128        # token chunks
    TW = T // 2          # half-of-tokens tile width (<= 512 for one PSUM bank)
    assert TW <= 512

    nc = bacc.Bacc("TRN2", target_bir_lowering=False, debug=False,
                   num_devices=n_devices)

    x_d = nc.declare_dram_parameter("x", [T, D], F32, isOutput=False)
    w_d = {}
    b_d = {}
    for nm in ("Wq", "Wk", "Wv", "W0", "W1", "W2"):
        w_d[nm] = nc.declare_dram_parameter(nm, [D, D], F32, isOutput=False)
    for nm in ("bq", "bk", "bv", "b0", "b1", "b2", "g1", "be1", "g2", "be2"):
        b_d[nm] = nc.declare_dram_parameter(nm, [D], F32, isOutput=False)
    out_d = nc.declare_dram_parameter("out", [T, D], F32, isOutput=True)
    dbg = {}
    if debug_taps:
        for nm, shape in [("xnT", [128, D // 128, T]), ("qT", [128, D // 128, T]),
                          ("kT", [128, D // 128, T]), ("vpad", [128, T // 128, (D // 64) * 65]),
                          ("y_tm", [128, T // 128, D // 128, 128]), ("rs", [16, T]),
                          ("x2", [T, D]), ("xn2T", [128, D // 128, T]),
                          ("h1T", [128, D // 128, T]), ("h3T", [128, D // 128, T])]:
            dbg[nm] = nc.declare_dram_parameter("dbg_" + nm, shape, F32, isOutput=True)

    with tile.TileContext(nc) as tc:
        with (
            tc.tile_pool(name="wp", bufs=2) as wp,            # (128, NF, D) f16 weights
            tc.tile_pool(name="bigp", bufs=4) as bigp,        # (128, NF, T)-ish f16
            tc.tile_pool(name="vp", bufs=1) as vp,            # vpad
            tc.tile_pool(name="ep", bufs=3) as ep,            # e^T chunks
            tc.tile_pool(name="up", bufs=2) as up,            # token-major u tiles
            tc.tile_pool(name="yp", bufs=3) as yp,            # yT pair tiles
            tc.tile_pool(name="xp", bufs=2) as xp,            # x fp32 tiles
            tc.tile_pool(name="x2p", bufs=NT) as x2p,         # x2 f16 tiles (all live)
            tc.tile_pool(name="scrp", bufs=2) as scrp,        # ACT accum scratch
            tc.tile_pool(name="outp", bufs=2) as outp,        # out fp32 staging
            tc.tile_pool(name="smallp", bufs=24) as smallp,   # stats & misc small
            tc.tile_pool(name="constp", bufs=1) as constp,    # per-partition consts
            tc.tile_pool(name="psp", bufs=2, space="PSUM") as psp,
        ):
            # x loads first: they gate the LN1 -> QKV critical path
            x_tiles = []
            for i in range(NT):
                xi = xp.tile([128, D], F32, tag="x", name=f"x_{i}")
                nc.sync.dma_start(xi[:], x_d[i * 128:(i + 1) * 128, :])
                x_tiles.append(xi)

            def dump(nm, src_ap):
                if not debug_taps:
                    return
                flat_n = 1
                for d_ in src_ap.shape[1:]:
                    flat_n *= d_
                stg = smallp.tile([src_ap.shape[0], flat_n], F32, tag="dump_" + nm,
                                  name="dump_" + nm, bufs=1)
                flat_src = src_ap
                if len(src_ap.shape) == 3:
                    flat_src = src_ap.rearrange("p a b -> p (a b)")
                elif len(src_ap.shape) == 4:
                    flat_src = src_ap.rearrange("p a b c -> p (a b c)")
                nc.vector.tensor_copy(stg[:], flat_src)
                dst = dbg[nm].ap()
                if len(dst.shape) == 3:
                    dst = dst.rearrange("p a b -> p (a b)")
                elif len(dst.shape) == 4:
                    dst = dst.rearrange("p a b c -> p (a b c)")
                nc.sync.dma_start(dst, stg[:])

            # ---------------- constant / vector loads ----------------
            bsb = {}
            for nm in ("bq", "bk", "b0", "b1", "b2", "g1", "be1", "g2", "be2"):
                t = constp.tile([128, NF], F32, tag=nm)
                nc.gpsimd.dma_start(t[:], b_d[nm].ap().rearrange("(c p) -> p c", p=128))
                bsb[nm] = t
            # bv broadcast across partitions (token-major use)
            bv_row = constp.tile([1, D], F32, tag="bv_row")
            nc.sync.dma_start(bv_row[:], b_d["bv"].ap().rearrange("(o d) -> o d", o=1))
            bvb = constp.tile([128, D], F32, tag="bvb")
            nc.gpsimd.partition_broadcast(bvb[:], bv_row[:])
            # scalar bias constants for activations
            eps_t = constp.tile([128, 1], F32, tag="eps")
            nc.gpsimd.memset(eps_t[:], EPS)
            nls_t = constp.tile([128, 1], F32, tag="nls")
            nc.gpsimd.memset(nls_t[:], -LOG_SHIFT)

            # ---------------- weight loads (f32 -> f16 SWDGE cast) ----------------
            def load_weight(nm):
                t = wp.tile([128, NF, D], F16, tag="w")
                nc.gpsimd.dma_start(t[:], w_d[nm].ap().rearrange("(c p) n -> p c n", p=128))
                return t

            wq_sb = load_weight("Wq")
            wk_sb = load_weight("Wk")

            # ---------------- LayerNorm helper ----------------
            def layer_norm_to_uT(x_tiles, uT_big, g_t, be_t, xnT_big):
                """x_tiles: NT token-major tiles -> xnT_big (128, NF, T) f16."""
                for i in range(NT):
                    xi = x_tiles[i]
                    scr = scrp.tile([128, D], F16, tag="scr")
                    ssum = smallp.tile([128, 1], F32, tag="st")
                    nc.scalar.activation(scr[:], xi[:], AF.Identity, accum_out=ssum[:])
                    scr2 = scrp.tile([128, D], F16, tag="scr")
                    ssq = smallp.tile([128, 1], F32, tag="st")
                    nc.scalar.activation(scr2[:], xi[:], AF.Square, accum_out=ssq[:])
                    mean = smallp.tile([128, 1], F32, tag="st")
                    nc.vector.tensor_scalar_mul(mean[:], ssum[:], 1.0 / D)
                    m2 = smallp.tile([128, 1], F32, tag="st")
                    nc.vector.tensor_tensor(out=m2[:], in0=mean[:], in1=mean[:], op=OP.mult)
                    v0 = smallp.tile([128, 1], F32, tag="st")
                    nc.vector.tensor_scalar_mul(v0[:], ssq[:], 1.0 / D)
                    var = smallp.tile([128, 1], F32, tag="st")
                    nc.vector.tensor_tensor(out=var[:], in0=v0[:], in1=m2[:], op=OP.subtract)
                    std = smallp.tile([128, 1], F32, tag="st")
                    nc.scalar.activation(std[:], var[:], AF.Sqrt, bias=eps_t[:])
                    rstd = smallp.tile([128, 1], F32, tag="st")
                    nc.vector.reciprocal(rstd[:], std[:])
                    u = up.tile([128, D], F16, tag="u")
                    nc.vector.tensor_scalar(out=u[:], in0=xi[:], scalar1=mean[:],
                                            scalar2=rstd[:], op0=OP.subtract, op1=OP.mult)
                    nc.sync.dma_start_transpose(uT_big[:, :, i * 128:(i + 1) * 128], u[:])
                # apply gain/bias per (feature chunk, token half): GEMM half th only
                # needs transposes of token chunks in that half
                for th in range(2):
                    for c in range(NF):
                        nc.vector.tensor_scalar(out=xnT_big[:, c, th * TW:(th + 1) * TW],
                                                in0=uT_big[:, c, th * TW:(th + 1) * TW],
                                                scalar1=g_t[:, c:c + 1], scalar2=be_t[:, c:c + 1],
                                                op0=OP.mult, op1=OP.add)

            # ---------------- LN1 ----------------
            uT_big = bigp.tile([128, NF, T], F16, tag="big")
            xnT_big = bigp.tile([128, NF, T], F16, tag="big")
            layer_norm_to_uT(x_tiles, uT_big, bsb["g1"], bsb["be1"], xnT_big)

            dump("xnT", xnT_big[:])

            # ---------------- QKV projections ----------------
            qT_big = bigp.tile([128, NF, T], F16, tag="big")
            kT_big = bigp.tile([128, NF, T], F16, tag="big")

            def proj_feature_major(w_sb, bias_t, dst_big, m, src_big):
                ps = psp.tile([128, 2 * TW], F32, tag="mm", name=f"proj_ps_{m}")
                for th in range(2):
                    for kc in range(NF):
                        nc.tensor.matmul(ps[:, th * TW:(th + 1) * TW],
                                         w_sb[:, kc, m * 128:(m + 1) * 128],
                                         src_big[:, kc, th * TW:(th + 1) * TW],
                                         start=(kc == 0), stop=(kc == NF - 1),
                                         skip_group_check=True)
                nc.vector.tensor_scalar(out=dst_big[:, m, :],
                                        in0=ps[:], scalar1=bias_t[:, m:m + 1],
                                        scalar2=None, op0=OP.add)

            # interleave K/Q so attention can start early
            for m in range(NF):
                proj_feature_major(wk_sb, bsb["bk"], kT_big, m, xnT_big)
                proj_feature_major(wq_sb, bsb["bq"], qT_big, m, xnT_big)

            wv_sb = load_weight("Wv")
            vpad = vp.tile([128, NT, H * 65], F16, tag="vpad")
            nc.vector.memset(vpad[:, :, :].rearrange("p c (h o) -> p c h o", o=65)[:, :, :, 64], 1.0)
            for i in range(NT):
                ps = psp.tile([128, D], F32, tag="mm", name=f"v_ps_{i}")
                for dh in range(2):
                    for kc in range(NF):
                        nc.tensor.matmul(ps[:, dh * (D // 2):(dh + 1) * (D // 2)],
                                         xnT_big[:, kc, i * 128:(i + 1) * 128],
                                         wv_sb[:, kc, dh * (D // 2):(dh + 1) * (D // 2)],
                                         start=(kc == 0), stop=(kc == NF - 1),
                                         skip_group_check=True)
                dst = vpad[:, i, :].rearrange("p (h o) -> p h o", o=65)[:, :, 0:64]
                nc.vector.tensor_tensor(
                    out=dst,
                    in0=ps[:].rearrange("p (h q) -> p h q", q=64),
                    in1=bvb[:].rearrange("p (h q) -> p h q", q=64),
                    op=OP.add)

            dump("qT", qT_big[:])
            dump("kT", kT_big[:])
            dump("vpad", vpad[:])

            # prefetch MLP weights during attention
            w0_sb = load_weight("W0")

            # ---------------- attention ----------------
            rs_sb = smallp.tile([16, T], F16, tag="rs", bufs=1)
            if H < 16:
                nc.gpsimd.memset(rs_sb[:], 1.0)
            y_tm_big = bigp.tile([128, NT, NF, 128], F16, tag="big")
            for d in range(NF):
                y_ps = [[psp.tile([65, TW], F32, tag="y", name=f"y_ps_{d}_{hh}_{th}", bufs=4)
                         for th in range(2)] for hh in range(2)]
                for s in range(NT):
                    e_t = [ep.tile([128, T], F16, tag="e", name=f"e_{d}_{s}_{hh}") for hh in range(2)]
                    for hh, base in ((0, 0), (1, 64)):
                        sc = psp.tile([128, 2 * TW], F32, tag="mm", name=f"sc_{d}_{s}_{hh}")
                        for th in range(2):
                            nc.tensor.matmul(sc[:, th * TW:(th + 1) * TW],
                                             qT_big[base:base + 64, d, s * 128:(s + 1) * 128],
                                             kT_big[base:base + 64, d, th * TW:(th + 1) * TW],
                                             start=True, stop=True,
                                             tile_position=(base, 0),
                                             skip_group_check=True)
                        nc.scalar.activation(e_t[hh][:], sc[:],
                                             AF.Exp, bias=nls_t[:], scale=0.125)
                    for hh in range(2):
                        h = 2 * d + hh
                        for th in range(2):
                            nc.tensor.matmul(y_ps[hh][th][:],
                                             vpad[:, s, h * 65:h * 65 + 65],
                                             e_t[hh][:, th * TW:(th + 1) * TW],
                                             start=(s == 0), stop=(s == NT - 1))
                # extract rowsums + y^T
                yT_d = yp.tile([128, T], F16, tag="yT")
                for hh in range(2):
                    h = 2 * d + hh
                    for th in range(2):
                        stg = smallp.tile([1, TW], F16, tag="stg", bufs=4)
                        nc.vector.tensor_copy(stg[:], y_ps[hh][th][64:65, :])
                        nc.sync.dma_start(rs_sb[h:h + 1, th * TW:(th + 1) * TW], stg[:])
                        nc.vector.tensor_copy(yT_d[hh * 64:(hh + 1) * 64, th * TW:(th + 1) * TW],
                                              y_ps[hh][th][0:64, :])
                nc.sync.dma_start_transpose(y_tm_big[:, :, d, :], yT_d[:])

            dump("y_tm", y_tm_big[:])
            dump("rs", rs_sb[:])
            rsT = smallp.tile([128, NT, 16], F16, tag="rsT", bufs=1)
            nc.sync.dma_start_transpose(rsT[:], rs_sb[:])

            # ---------------- residual + LN2 ----------------
            x2_tiles = []
            uT2_big = bigp.tile([128, NF, T], F16, tag="big")
            xn2T_big = bigp.tile([128, NF, T], F16, tag="big")
            for i in range(NT):
                xi = xp.tile([128, D], F32, tag="x")
                nc.sync.dma_start(xi[:], x_d[i * 128:(i + 1) * 128, :])
                recip = smallp.tile([128, 16], F32, tag="recip", bufs=2)
                nc.vector.reciprocal(recip[:, 0:H], rsT[:, i, 0:H])
                rfull = smallp.tile([128, H, 64], F32, tag="rfull", bufs=2)
                rsrc = recip[:, 0:H].rearrange("p (h o) -> p h o", o=1)
                rsrc = bass.AP(rsrc.tensor, rsrc.offset, [rsrc.ap[0], rsrc.ap[1], [0, 64]])
                nc.vector.tensor_copy(rfull[:], rsrc)
                ysc = outp.tile([128, D], F16, tag="ysc", bufs=2)
                nc.vector.tensor_tensor(out=ysc[:],
                                        in0=y_tm_big[:, i, :, :].rearrange("p c q -> p (c q)"),
                                        in1=rfull[:].rearrange("p h o -> p (h o)"),
                                        op=OP.mult)
                x2i = x2p.tile([128, D], F16, tag="x2")
                nc.vector.tensor_tensor(out=x2i[:], in0=xi[:], in1=ysc[:], op=OP.add)
                x2_tiles.append(x2i)
            if debug_taps:
                for i in range(NT):
                    stgx = smallp.tile([128, D], F32, tag="dump_x2", name=f"dump_x2_{i}", bufs=2)
                    nc.vector.tensor_copy(stgx[:], x2_tiles[i][:])
                    nc.sync.dma_start(dbg["x2"][i * 128:(i + 1) * 128, :], stgx[:])
            layer_norm_to_uT(x2_tiles, uT2_big, bsb["g2"], bsb["be2"], xn2T_big)

            # ---------------- MLP ----------------
            h1T_big = bigp.tile([128, NF, T], F16, tag="big")
            h2T_big = bigp.tile([128, NF, T], F16, tag="big")
            h3T_big = bigp.tile([128, NF, T], F16, tag="big")

            def mlp_layer(w_sb, bias_t, src_big, dst_big, relu, lname=""):
                for m in range(NF):
                    ps = psp.tile([128, 2 * TW], F32, tag="mm", name=f"mlp_ps_{lname}_{m}")
                    for th in range(2):
                        for kc in range(NF):
                            nc.tensor.matmul(ps[:, th * TW:(th + 1) * TW],
                                             w_sb[:, kc, m * 128:(m + 1) * 128],
                                             src_big[:, kc, th * TW:(th + 1) * TW],
                                             start=(kc == 0), stop=(kc == NF - 1),
                                             skip_group_check=True)
                    if relu:
                        nc.vector.tensor_scalar(out=dst_big[:, m, :],
                                                in0=ps[:], scalar1=bias_t[:, m:m + 1],
                                                scalar2=0.0, op0=OP.add, op1=OP.max)
                    else:
                        nc.vector.tensor_scalar(out=dst_big[:, m, :],
                                                in0=ps[:], scalar1=bias_t[:, m:m + 1],
                                                scalar2=None, op0=OP.add)

            w1_sb = load_weight("W1")
            mlp_layer(w0_sb, bsb["b0"], xn2T_big, h1T_big, relu=True, lname="l0")
            w2_sb = load_weight("W2")
            mlp_layer(w1_sb, bsb["b1"], h1T_big, h2T_big, relu=True, lname="l1")
            mlp_layer(w2_sb, bsb["b2"], h2T_big, h3T_big, relu=False, lname="l2")

            dump("xn2T", xn2T_big[:])
            dump("h1T", h1T_big[:])
            dump("h3T", h3T_big[:])
            h3_tm_big = bigp.tile([128, NT, NF, 128], F16, tag="big")
            for m in range(NF):
                nc.sync.dma_start_transpose(h3_tm_big[:, :, m, :], h3T_big[:, m, :])

            # ---------------- final residual + store ----------------
            for i in range(NT):
                osb = outp.tile([128, D], F32, tag="osb")
                nc.vector.tensor_tensor(out=osb[:], in0=x2_tiles[i][:],
                                        in1=h3_tm_big[:, i, :, :].rearrange("p c q -> p (c q)"),
                                        op=OP.add)
                nc.sync.dma_start(out_d[i * 128:(i + 1) * 128, :], osb[:])

    nc.compile()
    return nc


_NC_CACHE = {}


def _get_nc():
    if "nc" not in _NC_CACHE:
        _NC_CACHE["nc"] = build_attention_block(1024, 1024, 8)
    return _NC_CACHE["nc"]


def kernel(**inputs):
    nc = _get_nc()
    names = ["Wq", "bq", "Wk", "bk", "Wv", "bv", "g1", "be1", "g2", "be2",
             "W0", "b0", "W1", "b1", "W2", "b2"]
    shared = {nm: np.ascontiguousarray(np.asarray(inputs[nm], dtype=np.float32))
              for nm in names}
    x = np.asarray(inputs["x"], dtype=np.float32)
    in_maps = [dict(shared, x=np.ascontiguousarray(x[b])) for b in range(B)]
    res = run_bass_kernel_spmd(nc, in_maps, core_ids=list(range(B)))
    return np.stack([res.results[b]["out"] for b in range(B)], axis=0)
